# revision 41
# baseline (speedup 1.0000x reference)
"""Distributed Trainium2 kernel for a single causal attention head.

Problem (hardcoded): B=4, S=2048, D_MODEL=1024, HEAD_DIM=64, fp32 inputs.
    q = query @ Wq + bq ; k = key @ Wk + bk ; v = value @ Wv + bv
    scores = q k^T / sqrt(H) ; masked softmax ; out = att @ v

Sharding (8 NeuronCores): KEY-SPLIT partial softmax.  Core c = (b, h)
with b = c//2, h = c%2.  Each core handles ALL 2048 query rows of its
batch but only HALF of the keys: h owns global key j-tiles {h, h+2,
h+4, ..., h+14} (tile = 128 keys; even/odd interleave).  Query pair p
(512 rows, chunks 2p,2p+1) attends exactly local tiles 0..2p+1 (even
local tiles attended by both chunks = "wide", odd local boundary tile
by chunk 2p+1 only = "solo"), so one SPMD program serves all cores.
With the even/odd interleave every masked tile on a core uses the SAME
intra-tile predicate (allowed iff qcol >= p + 128*h), shipped from the
host as a single [128, 512] bf16 multiplicative mask.

Each core computes UNNORMALIZED partials: po[0:64, i] = sum_j att*v,
po[64, i] = sum_j att (denominator, via an appended ones row in v_aug).
The raw [65, 2048] fp32 partials are DMAed out and the host combines
(numA+numB)/(denA+denB) per batch.

PE layout: all matmuls contract over the partition dim:
  qT/kT/vT[h,:] = W^T X^T   (col-group-paired projection matmuls)
  v[j,h]        = vT via PE-transpose, ones col appended -> row 64
  sT            = ROW-PAIRED: two K=64 matmuls run concurrently in the
                  upper/lower halves of the PE array (tile_position
                  (0,0)/(64,0)): even local tile's kT lives in SBUF
                  partitions 0-63, odd tile's kT in 64-127, and q is
                  duplicated into both halves.  One 512-col pass
                  computes TWO score tiles -> 1.8x score throughput vs
                  zero-padded K=128 matmuls.
  att           = exp(sT) (ScalarE, one act spanning the round's two
                  PSUM banks, PSUM->SBUF bf16); causal mask applied by
                  GpSimd multiply on boundary rounds only
  po[65,i]     += v_aug-tile as lhsT, rhs=att
Engine split: ScalarE = exps (+4 partition-crossing proj copies each
for k,v); VectorE = projection dequant/bias + po stage copies; GpSimd =
q-duplication copies + causal-mask muls + identity; outputs on SWDGE.
DMA: weights FIRST on the sync ring, then k0, q0, q1, v0, q2, v1, q3,
k1, v2, v3 -- ordered so PE work unlocks just-in-time and the tail
(last avs + epilogue) is minimal.
"""

import os

import numpy as np
import ml_dtypes

import concourse.bass as bass
import concourse.tile as tile
from concourse import bacc, mybir
from concourse.bass import ds
from concourse.bass_utils import run_bass_kernel_spmd
from concourse.masks import make_identity

B, S, D, H = 4, 2048, 1024, 64
P = 128
NCORES = 8
CHUNK = 256               # query rows per chunk
NQ = S                    # every core sees all 2048 query rows
SL = S // 2               # local keys per core (1024)
JTL = SL // P             # 8 local j-tiles
NPAIRS = 4                # pairs of 512 query rows
NROUNDS = 4               # score rounds (pairs of local j-tiles)
DCH = D // P              # 8 contraction chunks
FP = mybir.dt.float32
BF = mybir.dt.bfloat16
BF_NP = ml_dtypes.bfloat16

# pair p attends local tiles 0..2p+1; tile 2p+1 is the solo tile
KS_PAIRS = tuple((2 * p + 1, 2 * p + 2) for p in range(NPAIRS))

LAST_RESULTS = None
_PROGRAM_CACHE = {}


def _build_program():
    """Build the SPMD Bass program (identical on all 8 cores)."""
    nc = bacc.Bacc("TRN2", target_bir_lowering=False, debug=False,
                   num_devices=NCORES)

    # inputs pre-packed in TRANSFER UNITS: [:, u] is contiguous per
    # partition (8/4 KB DMA lines -> near-peak HBM rate per transfer)
    qT_d = nc.dram_tensor("qT", [P, 4, DCH, 512], BF,
                          kind="ExternalInput").ap()
    kT_d = nc.dram_tensor("kT", [P, 2, DCH, 512], BF,
                          kind="ExternalInput").ap()
    vT_d = nc.dram_tensor("vT", [P, 4, DCH, 256], BF,
                          kind="ExternalInput").ap()
    wall_d = nc.dram_tensor("wall", [P, DCH, 3 * H], BF,
                            kind="ExternalInput").ap()
    # biases replicated into both partition halves so every op can use
    # a bias slice whose base partition matches its input's
    ball_d = nc.dram_tensor("ball", [P, 3], FP, kind="ExternalInput").ap()
    mask_d = nc.dram_tensor("mask", [P, 512], BF,
                            kind="ExternalInput").ap()
    out_d = nc.dram_tensor("out", [H + 1, NPAIRS, 2 * CHUNK], FP,
                           kind="ExternalOutput").ap()

    with tile.TileContext(nc) as tc:
        with (
            tc.tile_pool(name="const", bufs=1) as const,
            tc.tile_pool(name="resident", bufs=1) as res,
            tc.tile_pool(name="attp", bufs=20) as attp,
            tc.tile_pool(name="outp", bufs=2) as outp,
            tc.tile_pool(name="psc", bufs=4, space="PSUM") as psc,
            tc.tile_pool(name="pout", bufs=4, space="PSUM") as pout,
        ):
            # ---- weights FIRST on the sync ring (big-line DMA), tiny
            # bias vector + mask on the scalar ring ----
            wall_sb = const.tile([P, DCH, 3 * H], BF, tag="wall")
            nc.sync.dma_start(wall_sb, wall_d)
            ball_sb = const.tile([P, 3], FP, tag="ball")
            nc.scalar.dma_start(ball_sb, ball_d)
            mask_sb = const.tile([P, 512], BF, tag="mask")
            nc.scalar.dma_start(mask_sb, mask_d)
            wk_sb = wall_sb[:, :, 0:H]
            wv_sb = wall_sb[:, :, H:2 * H]
            wq_sb = wall_sb[:, :, 2 * H:3 * H]

            def bias(col, base):
                return ball_sb[base:base + H, col:col + 1]
            zeros_sb = const.tile([P, 2 * CHUNK], BF, tag="zeros")
            nc.vector.memset(zeros_sb, 0.0)
            identb = const.tile([P, P], BF, tag="identb")
            make_identity(nc, identb)

            # ---- big input DMAs, ONE ring (sync), arrival order
            # matched to the compute schedule below.  v units interleave
            # with q so av/transpose work overlaps the stream; k1 lands
            # before q3 so the late score rounds' exps overlap the final
            # v transfers and the kernel tail is only the last avs +
            # epilogue ----
            xk_sb = res.tile([P, 2, DCH, 512], BF, tag="xk")
            xv_sb = res.tile([P, 4, DCH, 256], BF, tag="xv")
            xq_sb = res.tile([P, 4, DCH, 512], BF, tag="xq")

            def dma_unit(dst, src, u):
                nc.sync.dma_start(dst[:, u], src[:, u])

            # REVERSED pair order: q3 (the pair with the longest
            # round/exp/av chain) lands first so its chain overlaps the
            # whole stream; the v units drive a dense av end-phase
            dma_unit(xk_sb, kT_d, 0)
            dma_unit(xq_sb, qT_d, 3)
            dma_unit(xq_sb, qT_d, 2)
            dma_unit(xk_sb, kT_d, 1)
            dma_unit(xq_sb, qT_d, 1)
            dma_unit(xv_sb, vT_d, 0)
            dma_unit(xq_sb, qT_d, 0)
            dma_unit(xv_sb, vT_d, 1)
            dma_unit(xv_sb, vT_d, 2)
            dma_unit(xv_sb, vT_d, 3)

            # ---- resident projected tensors ----
            # k_sb[0:64, g, :]  = kT of even local tile 2g
            # k_sb[64:128, g, :] = kT of odd  local tile 2g+1
            k_sb = res.tile([P, NROUNDS, P], BF, tag="k")
            # q duplicated into both partition halves for row-pairing
            q_sb = res.tile([P, NQ], BF, tag="q")
            vT_sb = res.tile([P, SL], BF, tag="vT")
            v_sb = res.tile([P, JTL, H + 1], BF, tag="v")
            nc.vector.memset(v_sb[:, :, H:], 1.0)  # softmax denom row

            # ---- PE warm-up: ramp the HAM clock until k0 lands.  Each
            # warm allocates a fresh rotating psum tile so it never
            # aliases a live accumulation ----
            def warm(n=1):
                for _ in range(n):
                    pw = psc.tile([P, 2 * CHUNK], FP, tag="sc",
                                  name="pwarm")
                    nc.tensor.matmul(pw, lhsT=zeros_sb[:, 0:P],
                                     rhs=zeros_sb, start=True, stop=True)

            warm(8)

            # col-group-paired projection: two M=64 matmuls concurrently
            # contract the same weight over two width/2-wide input chunks.
            def proj_pair(w_sb, x3, width, out_fn, name):
                hw = width // 2
                pj = psc.tile([P, 2 * CHUNK], FP, tag="sc", name=name)
                for d in range(DCH):
                    nc.tensor.matmul(pj[0:H, 0:hw], lhsT=w_sb[:, d, :],
                                     rhs=x3[:, d, ds(0, hw)],
                                     start=(d == 0), stop=(d == DCH - 1),
                                     skip_group_check=True)
                    nc.tensor.matmul(pj[H:2 * H, 0:hw], lhsT=w_sb[:, d, :],
                                     rhs=x3[:, d, ds(hw, hw)],
                                     start=(d == 0), stop=(d == DCH - 1),
                                     tile_position=(0, H),
                                     skip_group_check=True)
                out_fn(pj)

            # k unit u holds local keys [512u, 512u+512) = tiles 4u..4u+3
            def k_out(u, cross_eng):
                def fn(pj):
                    for m in range(4):          # local tile 4u+m
                        jt = 4 * u + m
                        g = jt // 2
                        sb = (m // 2) * H
                        src = pj[sb:sb + H, ds((m % 2) * P, P)]
                        dst = (k_sb[0:H, g, :] if jt % 2 == 0
                               else k_sb[H:2 * H, g, :])
                        if (m // 2) == (jt % 2):
                            nc.vector.tensor_scalar_add(dst, src,
                                                        bias(0, sb))
                        elif cross_eng == "scalar":
                            # k0's partition-crossing copies ride the
                            # then-idle ScalarE; k1's would queue behind
                            # the exp backlog, so they go to DVE
                            nc.scalar.activation(
                                dst, src,
                                mybir.ActivationFunctionType.Identity,
                                bias=bias(0, sb))
                        else:
                            nc.vector.tensor_scalar_add(dst, src,
                                                        bias(0, sb))
                return fn

            # q unit u covers query cols [512u, 512u+512)
            def q_out(u):
                c0 = 512 * u

                def fn(pj):
                    lo = q_sb[0:H, ds(c0, CHUNK)]
                    hi = q_sb[H:2 * H, ds(c0 + CHUNK, CHUNK)]
                    nc.vector.tensor_scalar_add(lo, pj[0:H, 0:CHUNK],
                                                bias(2, 0))
                    nc.vector.tensor_scalar_add(hi, pj[H:2 * H, 0:CHUNK],
                                                bias(2, H))
                    # duplicate into the other partition half (same
                    # engine -> in-order, no cross-engine semaphore)
                    nc.vector.tensor_copy(q_sb[H:2 * H, ds(c0, CHUNK)], lo)
                    nc.vector.tensor_copy(q_sb[0:H, ds(c0 + CHUNK, CHUNK)],
                                          hi)
                return fn

            # v unit u covers local v cols [256u, 256u+256) = tiles 2u,2u+1
            def v_out(u):
                c0 = 256 * u

                def fn(pj):
                    nc.vector.tensor_scalar_add(
                        vT_sb[0:H, ds(c0, P)], pj[0:H, 0:P], bias(1, 0))
                    nc.vector.tensor_scalar_add(
                        vT_sb[0:H, ds(c0 + P, P)], pj[H:2 * H, 0:P],
                        bias(1, H))
                    for jt in (2 * u, 2 * u + 1):
                        pvt = psc.tile([P, P], BF, tag="sc", name="pvt")
                        nc.tensor.transpose(pvt, vT_sb[:, ds(jt * P, P)],
                                            identb)
                        nc.vector.tensor_copy(v_sb[:, jt, 0:H], pvt[:, :H])
                return fn

            W = 2 * CHUNK  # 512
            att_tiles = {}
            po_tiles = {}
            mask_pending = {}

            # score round r of pair pr: local tiles (2r, 2r+1) computed
            # CONCURRENTLY in the lower/upper PE row-halves.
            def emit_round(pr, r):
                diag = (r == pr)
                nb = CHUNK if diag else W      # odd tile solo on diag
                c0 = pr * W
                c0b = c0 + (CHUNK if diag else 0)
                ra = psc.tile([P, W], FP, tag="sc", name="ra")
                rb = psc.tile([P, W], FP, tag="sc", name="rb")
                nc.tensor.matmul(ra, lhsT=k_sb[0:H, r, :],
                                 rhs=q_sb[0:H, ds(c0, W)],
                                 start=True, stop=True)
                nc.tensor.matmul(rb[:, 0:nb], lhsT=k_sb[H:2 * H, r, :],
                                 rhs=q_sb[H:2 * H, ds(c0b, nb)],
                                 start=True, stop=True)
                att = attp.tile([P, 2 * W], BF, tag="att", name="att")
                # k pre-scaled by 1/8 on host -> scores need no scale
                nc.scalar.activation(att[:, 0:W], ra,
                                     mybir.ActivationFunctionType.Exp)
                nc.scalar.activation(att[:, W:W + nb], rb[:, 0:nb],
                                     mybir.ActivationFunctionType.Exp)
                if diag:
                    mask_pending[pr] = (att, nb)
                att_tiles[(pr, 2 * r)] = (att[:, 0:W], c0, W)
                att_tiles[(pr, 2 * r + 1)] = (att[:, W:W + nb], c0b, nb)

            # causal-mask the boundary round of pair pr; deferred until
            # just before its avs so no engine FIFO is head-of-line
            # blocked waiting on the exp.  Off-tail masks ride the
            # otherwise-idle GpSimd; the last one rides the faster DVE.
            def emit_mask(pr, eng=None):
                eng = eng or nc.gpsimd
                att, nb = mask_pending.pop(pr)
                eng.tensor_mul(att[:, 0:W], att[:, 0:W], mask_sb[:, 0:W])
                eng.tensor_mul(att[:, W:W + nb], att[:, W:W + nb],
                               mask_sb[:, 0:nb])

            def emit_av(pr, jt):
                solo = KS_PAIRS[pr][1]
                if pr not in po_tiles:
                    po_tiles[pr] = pout.tile([H + 1, W], FP, tag="po",
                                             name=f"po{pr}")
                att, c0, n = att_tiles.pop((pr, jt))
                nc.tensor.matmul(po_tiles[pr][:, ds(c0 - pr * W, n)],
                                 lhsT=v_sb[:, jt, :], rhs=att,
                                 start=(jt == 0), stop=(jt == solo - 1),
                                 skip_group_check=True)

            def epilogue(pr):
                po = po_tiles[pr]
                stage = outp.tile([H + 1, W], FP, tag="stage")
                nc.vector.tensor_copy(stage, po)
                # HWDGE on the sync ring: the SWDGE descriptor build
                # costs ~2.5us, far too slow for the kernel tail
                nc.sync.dma_start(out_d[:, pr, :], stage)

            # ---- emission schedule (arrival order: wall, k0, q3, q2,
            # k1, q1, v0, q0, v1, v2, v3): pairs in REVERSE order so the
            # deepest round/exp chain (pair 3) overlaps the whole
            # stream; by v3's arrival only its two last avs remain ----
            proj_pair(wk_sb, xk_sb[:, 0], 512, k_out(0, "scalar"), "pk0")
            warm(5)   # bridge PE idle until q3 arrives
            proj_pair(wq_sb, xq_sb[:, 3], 512, q_out(3), "pq3")
            emit_round(3, 0)
            emit_round(3, 1)
            proj_pair(wq_sb, xq_sb[:, 2], 512, q_out(2), "pq2")
            emit_round(2, 0)
            emit_round(2, 1)
            proj_pair(wk_sb, xk_sb[:, 1], 512, k_out(1, "vector"), "pk1")
            emit_round(3, 2)
            emit_round(3, 3)
            emit_round(2, 2)
            emit_mask(3)
            emit_mask(2)
            proj_pair(wq_sb, xq_sb[:, 1], 512, q_out(1), "pq1")
            emit_round(1, 0)
            emit_round(1, 1)
            emit_mask(1)
            proj_pair(wv_sb, xv_sb[:, 0], 256, v_out(0), "pv0")
            emit_av(3, 0)
            emit_av(3, 1)
            emit_av(2, 0)
            emit_av(2, 1)
            emit_av(1, 0)
            emit_av(1, 1)
            proj_pair(wq_sb, xq_sb[:, 0], 512, q_out(0), "pq0")
            emit_round(0, 0)
            emit_mask(0, eng=nc.vector)
            emit_av(0, 0)
            emit_av(0, 1)
            epilogue(0)
            proj_pair(wv_sb, xv_sb[:, 1], 256, v_out(1), "pv1")
            emit_av(3, 2)
            emit_av(3, 3)
            emit_av(2, 2)
            emit_av(2, 3)
            emit_av(1, 2)
            emit_av(1, 3)
            epilogue(1)
            proj_pair(wv_sb, xv_sb[:, 2], 256, v_out(2), "pv2")
            emit_av(3, 4)
            emit_av(3, 5)
            emit_av(2, 4)
            emit_av(2, 5)
            epilogue(2)
            proj_pair(wv_sb, xv_sb[:, 3], 256, v_out(3), "pv3")
            emit_av(3, 6)
            emit_av(3, 7)
            epilogue(3)

    nc.compile()
    return nc


def _pack(xT):
    """[D, cols] -> [128, D/128, cols]: one contiguous DMA line/partition."""
    d, s = xT.shape
    return np.ascontiguousarray(
        xT.reshape(DCH, P, s).transpose(1, 0, 2)).astype(BF_NP)


def _np_reference(query, key, value, mask, Wq, bq, Wk, bk, Wv, bv):
    q = query @ Wq + bq
    k = key @ Wk + bk
    v = value @ Wv + bv
    scores = np.einsum("bqh,bkh->bqk", q, k) / np.sqrt(np.float32(H))
    scores = np.where(mask, scores, np.float32(-1e9))
    scores -= scores.max(axis=-1, keepdims=True)
    e = np.exp(scores)
    att = e / e.sum(axis=-1, keepdims=True)
    return np.einsum("bqk,bkh->bqh", att, v).astype(np.float32)


def kernel(query, key, value, mask, Wq, bq, Wk, bk, Wv, bv):
    global LAST_RESULTS
    query = np.asarray(query, dtype=np.float32)
    key = np.asarray(key, dtype=np.float32)
    value = np.asarray(value, dtype=np.float32)
    mask = np.asarray(mask).astype(bool)
    Wq = np.asarray(Wq, dtype=np.float32)
    Wk = np.asarray(Wk, dtype=np.float32)
    Wv = np.asarray(Wv, dtype=np.float32)
    bq = np.asarray(bq, dtype=np.float32)
    bk = np.asarray(bk, dtype=np.float32)
    bv = np.asarray(bv, dtype=np.float32)

    tril = np.tril(np.ones((S, S), dtype=bool))
    if not all(np.array_equal(mask[b], tril) for b in range(B)):
        # non-causal masks never occur for this problem; fall back to an
        # exact host implementation rather than an untested device path
        return _np_reference(query, key, value, mask, Wq, bq, Wk, bk,
                             Wv, bv)

    if "rp" not in _PROGRAM_CACHE:
        _PROGRAM_CACHE["rp"] = _build_program()
    nc = _PROGRAM_CACHE["rp"]

    def packw(w):
        return np.ascontiguousarray(
            w.reshape(DCH, P, H).transpose(1, 0, 2)).astype(BF_NP)

    # weight layout must match the wall_sb slicing: wk | wv | wq.
    # k is pre-scaled by 1/8 so scores come out of the matmul pre-scaled
    # and the exp activation needs no scale parameter.
    wall_in = np.concatenate(
        [packw(Wk * 0.125), packw(Wv), packw(Wq)], axis=2)
    wall_in = np.ascontiguousarray(wall_in)
    ball_half = np.stack([bk * 0.125, bv, bq], axis=1).astype(np.float32)
    ball_in = np.ascontiguousarray(
        np.concatenate([ball_half, ball_half], axis=0))

    pvec = np.arange(P, dtype=np.float32)
    ivec = np.arange(512, dtype=np.float32)
    in_maps = []
    for c in range(NCORES):
        b, h = divmod(c, 2)
        # even/odd global-tile interleave: core h owns tiles h, h+2, ...
        jglob = list(range(h, 2 * JTL, 2))
        cols = np.concatenate(
            [np.arange(j * P, (j + 1) * P) for j in jglob])

        def units(full, w):
            n = full.shape[-1] // w
            return np.ascontiguousarray(np.stack(
                [full[:, :, w * u:w * (u + 1)] for u in range(n)], axis=1))

        kT = units(_pack(key[b][cols].T), 512)      # [P, 2, DCH, 512]
        vT = units(_pack(value[b][cols].T), 256)    # [P, 4, DCH, 256]
        qT = units(_pack(query[b].T), 512)          # [P, 4, DCH, 512]
        # causal predicate for every masked tile on this core:
        # allowed iff qcol >= p + 128*h
        mask_in = (ivec[None, :] >= pvec[:, None] + 128 * h).astype(BF_NP)
        im = {"qT": qT, "kT": kT, "vT": vT,
              "wall": wall_in, "ball": ball_in,
              "mask": np.ascontiguousarray(mask_in)}
        in_maps.append(im)

    results = run_bass_kernel_spmd(
        nc, in_maps, core_ids=list(range(NCORES)),
        trace=bool(os.environ.get("BASS_TRACE")),
    )
    LAST_RESULTS = results

    out = np.empty((B, S, H), dtype=np.float32)
    for b in range(B):
        oA = results.results[2 * b]["out"].reshape(H + 1, NQ)
        oB = results.results[2 * b + 1]["out"].reshape(H + 1, NQ)
        num = oA[:H] + oB[:H]
        den = oA[H] + oB[H]
        out[b] = (num / den).T
    return out


# revision 42
# speedup vs baseline: 1.4485x; 1.4485x over previous
"""Distributed Trainium2 kernel for a single causal attention head.

Problem (hardcoded): B=4, S=2048, D_MODEL=1024, HEAD_DIM=64, fp32 inputs.
    q = query @ Wq + bq ; k = key @ Wk + bk ; v = value @ Wv + bv
    scores = q k^T / sqrt(H) ; masked softmax ; out = att @ v

Sharding (8 NeuronCores): KEY-SPLIT partial softmax.  Core c = (b, h)
with b = c//2, h = c%2.  Each core handles ALL 2048 query rows of its
batch but only HALF of the keys: h owns global key j-tiles {h, h+2,
h+4, ..., h+14} (tile = 128 keys; even/odd interleave).  Query pair p
(512 rows, chunks 2p,2p+1) attends exactly local tiles 0..2p+1 (even
local tiles attended by both chunks = "wide", odd local boundary tile
by chunk 2p+1 only = "solo"), so one SPMD program serves all cores.
With the even/odd interleave every masked tile on a core uses the SAME
intra-tile predicate (allowed iff qcol >= p + 128*h), shipped from the
host as a single [128, 512] bf16 multiplicative mask.

Each core computes UNNORMALIZED partials: po[0:64, i] = sum_j att*v,
po[64, i] = sum_j att (denominator, via an appended ones row in v_aug).
The raw [65, 2048] fp32 partials are DMAed out and the host combines
(numA+numB)/(denA+denB) per batch.

PE layout: all matmuls contract over the partition dim:
  qT/kT/vT[h,:] = W^T X^T   (col-group-paired projection matmuls)
  v[j,h]        = vT via PE-transpose, ones col appended -> row 64
  sT            = ROW-PAIRED: two K=64 matmuls run concurrently in the
                  upper/lower halves of the PE array (tile_position
                  (0,0)/(64,0)): even local tile's kT lives in SBUF
                  partitions 0-63, odd tile's kT in 64-127, and q is
                  duplicated into both halves.  One 512-col pass
                  computes TWO score tiles -> 1.8x score throughput vs
                  zero-padded K=128 matmuls.
  att           = exp(sT) (ScalarE, one act spanning the round's two
                  PSUM banks, PSUM->SBUF bf16); causal mask applied by
                  GpSimd multiply on boundary rounds only
  po[65,i]     += v_aug-tile as lhsT, rhs=att
Engine split: ScalarE = exps (+4 partition-crossing proj copies each
for k,v); VectorE = projection dequant/bias + po stage copies; GpSimd =
q-duplication copies + causal-mask muls + identity; outputs on SWDGE.
DMA: weights FIRST on the sync ring, then k0, q0, q1, v0, q2, v1, q3,
k1, v2, v3 -- ordered so PE work unlocks just-in-time and the tail
(last avs + epilogue) is minimal.
"""

import os

import numpy as np
import ml_dtypes

import concourse.bass as bass
import concourse.tile as tile
from concourse import bacc, mybir
from concourse.bass import ds
from concourse.bass_utils import run_bass_kernel_spmd
from concourse.masks import make_identity

B, S, D, H = 4, 2048, 1024, 64
P = 128
NCORES = 8
CHUNK = 256               # query rows per chunk
NQ = S                    # every core sees all 2048 query rows
SL = S // 2               # local keys per core (1024)
JTL = SL // P             # 8 local j-tiles
NPAIRS = 4                # pairs of 512 query rows
NROUNDS = 4               # score rounds (pairs of local j-tiles)
DCH = D // P              # 8 contraction chunks
FP = mybir.dt.float32
BF = mybir.dt.bfloat16
BF_NP = ml_dtypes.bfloat16

# pair p attends local tiles 0..2p+1; tile 2p+1 is the solo tile
KS_PAIRS = tuple((2 * p + 1, 2 * p + 2) for p in range(NPAIRS))

LAST_RESULTS = None
_PROGRAM_CACHE = {}


def _build_program():
    """Build the SPMD Bass program (identical on all 8 cores)."""
    nc = bacc.Bacc("TRN2", target_bir_lowering=False, debug=False,
                   num_devices=NCORES)

    # inputs pre-packed in TRANSFER UNITS: [:, u] is contiguous per
    # partition (8/4 KB DMA lines -> near-peak HBM rate per transfer)
    qT_d = nc.dram_tensor("qT", [P, 4, DCH, 512], BF,
                          kind="ExternalInput").ap()
    kT_d = nc.dram_tensor("kT", [P, 2, DCH, 512], BF,
                          kind="ExternalInput").ap()
    vT_d = nc.dram_tensor("vT", [P, 4, DCH, 256], BF,
                          kind="ExternalInput").ap()
    wall_d = nc.dram_tensor("wall", [P, DCH, 3 * H], BF,
                            kind="ExternalInput").ap()
    # biases replicated into both partition halves so every op can use
    # a bias slice whose base partition matches its input's
    ball_d = nc.dram_tensor("ball", [P, 3], FP, kind="ExternalInput").ap()
    mask_d = nc.dram_tensor("mask", [P, 512], BF,
                            kind="ExternalInput").ap()
    out_d = nc.dram_tensor("out", [H + 1, NPAIRS, 2 * CHUNK], FP,
                           kind="ExternalOutput").ap()

    with tile.TileContext(nc) as tc:
        with (
            tc.tile_pool(name="const", bufs=1) as const,
            tc.tile_pool(name="resident", bufs=1) as res,
            tc.tile_pool(name="attp", bufs=20) as attp,
            tc.tile_pool(name="outp", bufs=2) as outp,
            tc.tile_pool(name="psr", bufs=2, space="PSUM") as psr,
            tc.tile_pool(name="psm", bufs=2, space="PSUM") as psm,
            tc.tile_pool(name="pout", bufs=2, space="PSUM") as pout,
        ):
            # ---- weights FIRST on the sync ring (big-line DMA), tiny
            # bias vector + mask on the scalar ring ----
            wall_sb = const.tile([P, DCH, 3 * H], BF, tag="wall")
            nc.sync.dma_start(wall_sb, wall_d)
            ball_sb = const.tile([P, 3], FP, tag="ball")
            nc.scalar.dma_start(ball_sb, ball_d)
            mask_sb = const.tile([P, 512], BF, tag="mask")
            nc.scalar.dma_start(mask_sb, mask_d)
            wk_sb = wall_sb[:, :, 0:H]
            wv_sb = wall_sb[:, :, H:2 * H]
            wq_sb = wall_sb[:, :, 2 * H:3 * H]

            def bias(col, base):
                return ball_sb[base:base + H, col:col + 1]
            zeros_sb = const.tile([P, 2 * CHUNK], BF, tag="zeros")
            nc.vector.memset(zeros_sb, 0.0)
            identb = const.tile([P, P], BF, tag="identb")
            make_identity(nc, identb)

            # ---- big input DMAs, ONE ring (sync), arrival order
            # matched to the compute schedule below.  v units interleave
            # with q so av/transpose work overlaps the stream; k1 lands
            # before q3 so the late score rounds' exps overlap the final
            # v transfers and the kernel tail is only the last avs +
            # epilogue ----
            xk_sb = res.tile([P, 2, DCH, 512], BF, tag="xk")
            xv_sb = res.tile([P, 4, DCH, 256], BF, tag="xv")
            xq_sb = res.tile([P, 4, DCH, 512], BF, tag="xq")

            def dma_unit(dst, src, u):
                nc.sync.dma_start(dst[:, u], src[:, u])

            dma_unit(xk_sb, kT_d, 0)
            dma_unit(xq_sb, qT_d, 0)
            dma_unit(xq_sb, qT_d, 1)
            dma_unit(xv_sb, vT_d, 0)
            dma_unit(xq_sb, qT_d, 2)
            dma_unit(xq_sb, qT_d, 3)
            dma_unit(xk_sb, kT_d, 1)
            dma_unit(xv_sb, vT_d, 1)
            dma_unit(xv_sb, vT_d, 2)
            dma_unit(xv_sb, vT_d, 3)

            # ---- resident projected tensors ----
            # k_sb[0:64, g, :]  = kT of even local tile 2g
            # k_sb[64:128, g, :] = kT of odd  local tile 2g+1
            k_sb = res.tile([P, NROUNDS, P], BF, tag="k")
            # q duplicated into both partition halves for row-pairing
            q_sb = res.tile([P, NQ], BF, tag="q")
            vT_sb = res.tile([P, SL], BF, tag="vT")
            v_sb = res.tile([P, JTL, H + 1], BF, tag="v")
            nc.vector.memset(v_sb[:, :, H:], 1.0)  # softmax denom row

            # ---- PE warm-up: ramp the HAM clock until k0 lands ----
            pwarm = psm.tile([P, 2 * CHUNK], FP, tag="ps", name="pwarm")

            def warm(n=1):
                for _ in range(n):
                    nc.tensor.matmul(pwarm, lhsT=zeros_sb[:, 0:P],
                                     rhs=zeros_sb, start=True, stop=True)

            warm(8)

            # col-group-paired projection: two M=64 matmuls concurrently
            # contract the same weight over two width/2-wide input chunks.
            def proj_pair(w_sb, x3, width, out_fn, name):
                hw = width // 2
                pj = psm.tile([P, 2 * CHUNK], FP, tag="ps", name=name)
                for d in range(DCH):
                    nc.tensor.matmul(pj[0:H, 0:hw], lhsT=w_sb[:, d, :],
                                     rhs=x3[:, d, ds(0, hw)],
                                     start=(d == 0), stop=(d == DCH - 1),
                                     skip_group_check=True)
                    nc.tensor.matmul(pj[H:2 * H, 0:hw], lhsT=w_sb[:, d, :],
                                     rhs=x3[:, d, ds(hw, hw)],
                                     start=(d == 0), stop=(d == DCH - 1),
                                     tile_position=(0, H),
                                     skip_group_check=True)
                out_fn(pj)

            # k unit u holds local keys [512u, 512u+512) = tiles 4u..4u+3
            def k_out(u, cross_eng):
                def fn(pj):
                    for m in range(4):          # local tile 4u+m
                        jt = 4 * u + m
                        g = jt // 2
                        sb = (m // 2) * H
                        src = pj[sb:sb + H, ds((m % 2) * P, P)]
                        dst = (k_sb[0:H, g, :] if jt % 2 == 0
                               else k_sb[H:2 * H, g, :])
                        if (m // 2) == (jt % 2):
                            nc.vector.tensor_scalar_add(dst, src,
                                                        bias(0, sb))
                        elif cross_eng == "scalar":
                            # k0's partition-crossing copies ride the
                            # then-idle ScalarE; k1's would queue behind
                            # the exp backlog, so they go to DVE
                            nc.scalar.activation(
                                dst, src,
                                mybir.ActivationFunctionType.Identity,
                                bias=bias(0, sb))
                        else:
                            nc.vector.tensor_scalar_add(dst, src,
                                                        bias(0, sb))
                return fn

            # q unit u covers query cols [512u, 512u+512)
            def q_out(u):
                c0 = 512 * u

                def fn(pj):
                    lo = q_sb[0:H, ds(c0, CHUNK)]
                    hi = q_sb[H:2 * H, ds(c0 + CHUNK, CHUNK)]
                    nc.vector.tensor_scalar_add(lo, pj[0:H, 0:CHUNK],
                                                bias(2, 0))
                    nc.vector.tensor_scalar_add(hi, pj[H:2 * H, 0:CHUNK],
                                                bias(2, H))
                    # duplicate into the other partition half (same
                    # engine -> in-order, no cross-engine semaphore)
                    nc.vector.tensor_copy(q_sb[H:2 * H, ds(c0, CHUNK)], lo)
                    nc.vector.tensor_copy(q_sb[0:H, ds(c0 + CHUNK, CHUNK)],
                                          hi)
                return fn

            # v unit u covers local v cols [256u, 256u+256) = tiles 2u,2u+1
            def v_out(u):
                c0 = 256 * u

                def fn(pj):
                    nc.vector.tensor_scalar_add(
                        vT_sb[0:H, ds(c0, P)], pj[0:H, 0:P], bias(1, 0))
                    nc.vector.tensor_scalar_add(
                        vT_sb[0:H, ds(c0 + P, P)], pj[H:2 * H, 0:P],
                        bias(1, H))
                    for jt in (2 * u, 2 * u + 1):
                        pvt = psm.tile([P, P], BF, tag="ps", name="pvt")
                        nc.tensor.transpose(pvt, vT_sb[:, ds(jt * P, P)],
                                            identb)
                        nc.vector.tensor_copy(v_sb[:, jt, 0:H], pvt[:, :H])
                return fn

            W = 2 * CHUNK  # 512
            att_tiles = {}
            po_tiles = {}
            mask_pending = {}

            # score round r of pair pr: local tiles (2r, 2r+1) computed
            # CONCURRENTLY in the lower/upper PE row-halves.
            def emit_round(pr, r):
                diag = (r == pr)
                nb = CHUNK if diag else W      # odd tile solo on diag
                c0 = pr * W
                c0b = c0 + (CHUNK if diag else 0)
                width = W + nb
                rt = psr.tile([P, 2 * W], FP, tag="rt", name="rt")
                nc.tensor.matmul(rt[:, 0:W], lhsT=k_sb[0:H, r, :],
                                 rhs=q_sb[0:H, ds(c0, W)],
                                 start=True, stop=True)
                nc.tensor.matmul(rt[:, W:W + nb], lhsT=k_sb[H:2 * H, r, :],
                                 rhs=q_sb[H:2 * H, ds(c0b, nb)],
                                 start=True, stop=True)
                att = attp.tile([P, 2 * W], BF, tag="att", name="att")
                # k pre-scaled by 1/8 on host -> scores need no scale;
                # one exp spans the round's two PSUM banks
                nc.scalar.activation(att[:, 0:width], rt[:, 0:width],
                                     mybir.ActivationFunctionType.Exp)
                if diag:
                    mask_pending[pr] = (att, nb)
                att_tiles[(pr, 2 * r)] = (att[:, 0:W], c0, W)
                att_tiles[(pr, 2 * r + 1)] = (att[:, W:W + nb], c0b, nb)

            # causal-mask the boundary round of pair pr; deferred until
            # just before its avs so no engine FIFO is head-of-line
            # blocked waiting on the exp.  Off-tail masks ride the
            # otherwise-idle GpSimd; the last one rides the faster DVE.
            def emit_mask(pr, eng=None):
                eng = eng or nc.gpsimd
                att, nb = mask_pending.pop(pr)
                eng.tensor_mul(att[:, 0:W], att[:, 0:W], mask_sb[:, 0:W])
                eng.tensor_mul(att[:, W:W + nb], att[:, W:W + nb],
                               mask_sb[:, 0:nb])

            def emit_av(pr, jt):
                solo = KS_PAIRS[pr][1]
                if pr not in po_tiles:
                    po_tiles[pr] = pout.tile([H + 1, W], FP, tag="po",
                                             name=f"po{pr}")
                att, c0, n = att_tiles.pop((pr, jt))
                nc.tensor.matmul(po_tiles[pr][:, ds(c0 - pr * W, n)],
                                 lhsT=v_sb[:, jt, :], rhs=att,
                                 start=(jt == 0), stop=(jt == solo - 1),
                                 skip_group_check=True)

            def epilogue(pr):
                po = po_tiles[pr]
                stage = outp.tile([H + 1, W], FP, tag="stage")
                nc.vector.tensor_copy(stage, po)
                # HWDGE on the sync ring: the SWDGE descriptor build
                # costs ~2.5us, far too slow for the kernel tail
                nc.sync.dma_start(out_d[:, pr, :], stage)

            # ---- emission schedule (arrival order: wall, k0, q0, q1,
            # v0, q2, q3, k1, v1, v2, v3) ----
            proj_pair(wk_sb, xk_sb[:, 0], 512, k_out(0, "scalar"), "pk0")
            warm(5)   # bridge PE idle until q0 arrives
            proj_pair(wq_sb, xq_sb[:, 0], 512, q_out(0), "pq0")
            emit_round(0, 0)
            proj_pair(wq_sb, xq_sb[:, 1], 512, q_out(1), "pq1")
            emit_round(1, 0)
            emit_round(1, 1)
            emit_mask(0)
            proj_pair(wv_sb, xv_sb[:, 0], 256, v_out(0), "pv0")
            emit_av(0, 0)
            emit_av(0, 1)
            epilogue(0)
            emit_av(1, 0)
            emit_av(1, 1)
            proj_pair(wq_sb, xq_sb[:, 2], 512, q_out(2), "pq2")
            emit_round(2, 0)
            emit_round(2, 1)
            emit_av(2, 0)
            emit_av(2, 1)
            proj_pair(wq_sb, xq_sb[:, 3], 512, q_out(3), "pq3")
            emit_round(3, 0)
            emit_round(3, 1)
            emit_mask(1)
            proj_pair(wk_sb, xk_sb[:, 1], 512, k_out(1, "vector"), "pk1")
            emit_round(2, 2)
            emit_round(3, 2)
            emit_round(3, 3)
            proj_pair(wv_sb, xv_sb[:, 1], 256, v_out(1), "pv1")
            emit_av(1, 2)
            emit_av(1, 3)
            epilogue(1)
            emit_av(2, 2)
            emit_av(2, 3)
            emit_av(3, 0)
            emit_av(3, 1)
            emit_av(3, 2)
            emit_av(3, 3)
            emit_mask(2)
            proj_pair(wv_sb, xv_sb[:, 2], 256, v_out(2), "pv2")
            emit_av(2, 4)
            emit_av(2, 5)
            epilogue(2)
            emit_av(3, 4)
            emit_av(3, 5)
            emit_mask(3, eng=nc.vector)
            proj_pair(wv_sb, xv_sb[:, 3], 256, v_out(3), "pv3")
            emit_av(3, 6)
            emit_av(3, 7)
            epilogue(3)

    nc.compile()
    return nc


def _pack(xT):
    """[D, cols] -> [128, D/128, cols]: one contiguous DMA line/partition."""
    d, s = xT.shape
    return np.ascontiguousarray(
        xT.reshape(DCH, P, s).transpose(1, 0, 2)).astype(BF_NP)


def _np_reference(query, key, value, mask, Wq, bq, Wk, bk, Wv, bv):
    q = query @ Wq + bq
    k = key @ Wk + bk
    v = value @ Wv + bv
    scores = np.einsum("bqh,bkh->bqk", q, k) / np.sqrt(np.float32(H))
    scores = np.where(mask, scores, np.float32(-1e9))
    scores -= scores.max(axis=-1, keepdims=True)
    e = np.exp(scores)
    att = e / e.sum(axis=-1, keepdims=True)
    return np.einsum("bqk,bkh->bqh", att, v).astype(np.float32)


def kernel(query, key, value, mask, Wq, bq, Wk, bk, Wv, bv):
    global LAST_RESULTS
    query = np.asarray(query, dtype=np.float32)
    key = np.asarray(key, dtype=np.float32)
    value = np.asarray(value, dtype=np.float32)
    mask = np.asarray(mask).astype(bool)
    Wq = np.asarray(Wq, dtype=np.float32)
    Wk = np.asarray(Wk, dtype=np.float32)
    Wv = np.asarray(Wv, dtype=np.float32)
    bq = np.asarray(bq, dtype=np.float32)
    bk = np.asarray(bk, dtype=np.float32)
    bv = np.asarray(bv, dtype=np.float32)

    tril = np.tril(np.ones((S, S), dtype=bool))
    if not all(np.array_equal(mask[b], tril) for b in range(B)):
        # non-causal masks never occur for this problem; fall back to an
        # exact host implementation rather than an untested device path
        return _np_reference(query, key, value, mask, Wq, bq, Wk, bk,
                             Wv, bv)

    if "rp" not in _PROGRAM_CACHE:
        _PROGRAM_CACHE["rp"] = _build_program()
    nc = _PROGRAM_CACHE["rp"]

    def packw(w):
        return np.ascontiguousarray(
            w.reshape(DCH, P, H).transpose(1, 0, 2)).astype(BF_NP)

    # weight layout must match the wall_sb slicing: wk | wv | wq.
    # k is pre-scaled by 1/8 so scores come out of the matmul pre-scaled
    # and the exp activation needs no scale parameter.
    wall_in = np.concatenate(
        [packw(Wk * 0.125), packw(Wv), packw(Wq)], axis=2)
    wall_in = np.ascontiguousarray(wall_in)
    ball_half = np.stack([bk * 0.125, bv, bq], axis=1).astype(np.float32)
    ball_in = np.ascontiguousarray(
        np.concatenate([ball_half, ball_half], axis=0))

    pvec = np.arange(P, dtype=np.float32)
    ivec = np.arange(512, dtype=np.float32)
    in_maps = []
    for c in range(NCORES):
        b, h = divmod(c, 2)
        # even/odd global-tile interleave: core h owns tiles h, h+2, ...
        jglob = list(range(h, 2 * JTL, 2))
        cols = np.concatenate(
            [np.arange(j * P, (j + 1) * P) for j in jglob])

        def units(full, w):
            n = full.shape[-1] // w
            return np.ascontiguousarray(np.stack(
                [full[:, :, w * u:w * (u + 1)] for u in range(n)], axis=1))

        kT = units(_pack(key[b][cols].T), 512)      # [P, 2, DCH, 512]
        vT = units(_pack(value[b][cols].T), 256)    # [P, 4, DCH, 256]
        qT = units(_pack(query[b].T), 512)          # [P, 4, DCH, 512]
        # causal predicate for every masked tile on this core:
        # allowed iff qcol >= p + 128*h
        mask_in = (ivec[None, :] >= pvec[:, None] + 128 * h).astype(BF_NP)
        im = {"qT": qT, "kT": kT, "vT": vT,
              "wall": wall_in, "ball": ball_in,
              "mask": np.ascontiguousarray(mask_in)}
        in_maps.append(im)

    results = run_bass_kernel_spmd(
        nc, in_maps, core_ids=list(range(NCORES)),
        trace=bool(os.environ.get("BASS_TRACE")),
    )
    LAST_RESULTS = results

    out = np.empty((B, S, H), dtype=np.float32)
    for b in range(B):
        oA = results.results[2 * b]["out"].reshape(H + 1, NQ)
        oB = results.results[2 * b + 1]["out"].reshape(H + 1, NQ)
        num = oA[:H] + oB[:H]
        den = oA[H] + oB[H]
        out[b] = (num / den).T
    return out


# revision 45
# speedup vs baseline: 1.5615x; 1.0780x over previous
"""Distributed Trainium2 kernel for a single causal attention head.

Problem (hardcoded): B=4, S=2048, D_MODEL=1024, HEAD_DIM=64, fp32 inputs.
    q = query @ Wq + bq ; k = key @ Wk + bk ; v = value @ Wv + bv
    scores = q k^T / sqrt(H) ; masked softmax ; out = att @ v

Sharding (8 NeuronCores): KEY-SPLIT partial softmax.  Core c = (b, h)
with b = c//2, h = c%2.  Each core handles ALL 2048 query rows of its
batch but only HALF of the keys: h owns global key j-tiles {h, h+2,
h+4, ..., h+14} (tile = 128 keys; even/odd interleave).  Query pair p
(512 rows, chunks 2p,2p+1) attends exactly local tiles 0..2p+1 (even
local tiles attended by both chunks = "wide", odd local boundary tile
by chunk 2p+1 only = "solo"), so one SPMD program serves all cores.
With the even/odd interleave every masked tile on a core uses the SAME
intra-tile predicate (allowed iff qcol >= p + 128*h), shipped from the
host as a single [128, 512] bf16 multiplicative mask.

Each core computes UNNORMALIZED partials: po[0:64, i] = sum_j att*v,
po[64, i] = sum_j att (denominator, via an appended ones row in v_aug).
The raw [65, 2048] fp32 partials are DMAed out and the host combines
(numA+numB)/(denA+denB) per batch.

PE layout: all matmuls contract over the partition dim:
  qT/kT/vT[h,:] = W^T X^T   (col-group-paired projection matmuls)
  v[j,h]        = vT via PE-transpose, ones col appended -> row 64
  sT            = ROW-PAIRED: two K=64 matmuls run concurrently in the
                  upper/lower halves of the PE array (tile_position
                  (0,0)/(64,0)): even local tile's kT lives in SBUF
                  partitions 0-63, odd tile's kT in 64-127, and q is
                  duplicated into both halves.  One 512-col pass
                  computes TWO score tiles -> 1.8x score throughput vs
                  zero-padded K=128 matmuls.
  att           = exp(sT) (ScalarE, one act spanning the round's two
                  PSUM banks, PSUM->SBUF bf16); causal mask applied by
                  GpSimd multiply on boundary rounds only
  po[65,i]     += v_aug-tile as lhsT, rhs=att
Engine split: ScalarE = exps (+4 partition-crossing proj copies each
for k,v); VectorE = projection dequant/bias + po stage copies; GpSimd =
q-duplication copies + causal-mask muls + identity; outputs on SWDGE.
DMA: weights FIRST on the sync ring, then k0, q0, q1, v0, q2, q3, k1,
v1, v2, v3 -- ordered so PE work unlocks just-in-time and the tail
(last avs + epilogue) is minimal; epilogues leave via sync-ring HWDGE
(SWDGE descriptor builds are ~2.5us, too slow for the tail).
"""

import os

import numpy as np
import ml_dtypes

import concourse.bass as bass
import concourse.tile as tile
from concourse import bacc, mybir
from concourse.bass import ds
from concourse.bass_utils import run_bass_kernel_spmd
from concourse.masks import make_identity

B, S, D, H = 4, 2048, 1024, 64
P = 128
NCORES = 8
CHUNK = 256               # query rows per chunk
NQ = S                    # every core sees all 2048 query rows
SL = S // 2               # local keys per core (1024)
JTL = SL // P             # 8 local j-tiles
NPAIRS = 4                # pairs of 512 query rows
NROUNDS = 4               # score rounds (pairs of local j-tiles)
DCH = D // P              # 8 contraction chunks
FP = mybir.dt.float32
BF = mybir.dt.bfloat16
BF_NP = ml_dtypes.bfloat16

# pair p attends local tiles 0..2p+1; tile 2p+1 is the solo tile
KS_PAIRS = tuple((2 * p + 1, 2 * p + 2) for p in range(NPAIRS))

LAST_RESULTS = None
_PROGRAM_CACHE = {}


def _build_program():
    """Build the SPMD Bass program (identical on all 8 cores)."""
    nc = bacc.Bacc("TRN2", target_bir_lowering=False, debug=False,
                   num_devices=NCORES)

    # inputs pre-packed in TRANSFER UNITS: [:, u] is contiguous per
    # partition (8/4 KB DMA lines -> near-peak HBM rate per transfer)
    qT_d = nc.dram_tensor("qT", [P, 4, DCH, 512], BF,
                          kind="ExternalInput").ap()
    kT_d = nc.dram_tensor("kT", [P, 2, DCH, 512], BF,
                          kind="ExternalInput").ap()
    vT_d = nc.dram_tensor("vT", [P, 4, DCH, 256], BF,
                          kind="ExternalInput").ap()
    wall_d = nc.dram_tensor("wall", [P, DCH, 3 * H], BF,
                            kind="ExternalInput").ap()
    # biases replicated into both partition halves so every op can use
    # a bias slice whose base partition matches its input's
    ball_d = nc.dram_tensor("ball", [P, 3], FP, kind="ExternalInput").ap()
    mask_d = nc.dram_tensor("mask", [P, 512], BF,
                            kind="ExternalInput").ap()
    out_d = nc.dram_tensor("out", [H + 1, NPAIRS, 2 * CHUNK], FP,
                           kind="ExternalOutput").ap()

    with tile.TileContext(nc) as tc:
        with (
            tc.tile_pool(name="const", bufs=1) as const,
            tc.tile_pool(name="resident", bufs=1) as res,
            tc.tile_pool(name="attp", bufs=20) as attp,
            tc.tile_pool(name="outp", bufs=2) as outp,
            tc.tile_pool(name="psr", bufs=2, space="PSUM") as psr,
            tc.tile_pool(name="psm", bufs=2, space="PSUM") as psm,
            tc.tile_pool(name="pout", bufs=2, space="PSUM") as pout,
        ):
            # ---- weights FIRST on the sync ring (big-line DMA), tiny
            # bias vector + mask on the scalar ring ----
            wall_sb = const.tile([P, DCH, 3 * H], BF, tag="wall")
            nc.sync.dma_start(wall_sb, wall_d)
            ball_sb = const.tile([P, 3], FP, tag="ball")
            nc.scalar.dma_start(ball_sb, ball_d)
            mask_sb = const.tile([P, 512], BF, tag="mask")
            nc.scalar.dma_start(mask_sb, mask_d)
            wk_sb = wall_sb[:, :, 0:H]
            wv_sb = wall_sb[:, :, H:2 * H]
            wq_sb = wall_sb[:, :, 2 * H:3 * H]

            def bias(col, base):
                return ball_sb[base:base + H, col:col + 1]
            zeros_sb = const.tile([P, 2 * CHUNK], BF, tag="zeros")
            nc.vector.memset(zeros_sb, 0.0)
            identb = const.tile([P, P], BF, tag="identb")
            make_identity(nc, identb)

            # ---- big input DMAs, ONE ring (sync), arrival order
            # matched to the compute schedule below.  v units interleave
            # with q so av/transpose work overlaps the stream; k1 lands
            # before q3 so the late score rounds' exps overlap the final
            # v transfers and the kernel tail is only the last avs +
            # epilogue ----
            xk_sb = res.tile([P, 2, DCH, 512], BF, tag="xk")
            xv_sb = res.tile([P, 4, DCH, 256], BF, tag="xv")
            xq_sb = res.tile([P, 4, DCH, 512], BF, tag="xq")

            def dma_unit(dst, src, u):
                nc.sync.dma_start(dst[:, u], src[:, u])

            dma_unit(xk_sb, kT_d, 0)
            dma_unit(xq_sb, qT_d, 0)
            dma_unit(xq_sb, qT_d, 1)
            dma_unit(xv_sb, vT_d, 0)
            dma_unit(xq_sb, qT_d, 2)
            dma_unit(xq_sb, qT_d, 3)
            dma_unit(xk_sb, kT_d, 1)
            dma_unit(xv_sb, vT_d, 1)
            dma_unit(xv_sb, vT_d, 2)
            dma_unit(xv_sb, vT_d, 3)

            # ---- resident projected tensors ----
            # k_sb[0:64, g, :]  = kT of even local tile 2g
            # k_sb[64:128, g, :] = kT of odd  local tile 2g+1
            k_sb = res.tile([P, NROUNDS, P], BF, tag="k")
            # q duplicated into both partition halves for row-pairing
            q_sb = res.tile([P, NQ], BF, tag="q")
            vT_sb = res.tile([P, SL], BF, tag="vT")
            v_sb = res.tile([P, JTL, H + 1], BF, tag="v")
            nc.vector.memset(v_sb[:, :, H:], 1.0)  # softmax denom row

            # ---- PE warm-up: ramp the HAM clock until k0 lands ----
            pwarm = psm.tile([P, 2 * CHUNK], FP, tag="ps", name="pwarm")

            def warm(n=1):
                for _ in range(n):
                    nc.tensor.matmul(pwarm, lhsT=zeros_sb[:, 0:P],
                                     rhs=zeros_sb, start=True, stop=True)

            warm(8)

            # col-group-paired projection: two M=64 matmuls concurrently
            # contract the same weight over two width/2-wide input chunks.
            def proj_pair(w_sb, x3, width, out_fn, name):
                hw = width // 2
                pj = psm.tile([P, 2 * CHUNK], FP, tag="ps", name=name)
                for d in range(DCH):
                    nc.tensor.matmul(pj[0:H, 0:hw], lhsT=w_sb[:, d, :],
                                     rhs=x3[:, d, ds(0, hw)],
                                     start=(d == 0), stop=(d == DCH - 1),
                                     skip_group_check=True)
                    nc.tensor.matmul(pj[H:2 * H, 0:hw], lhsT=w_sb[:, d, :],
                                     rhs=x3[:, d, ds(hw, hw)],
                                     start=(d == 0), stop=(d == DCH - 1),
                                     tile_position=(0, H),
                                     skip_group_check=True)
                out_fn(pj)

            # k unit u holds local keys [512u, 512u+512) = tiles 4u..4u+3
            def k_out(u, cross_eng):
                def fn(pj):
                    for m in range(4):          # local tile 4u+m
                        jt = 4 * u + m
                        g = jt // 2
                        sb = (m // 2) * H
                        src = pj[sb:sb + H, ds((m % 2) * P, P)]
                        dst = (k_sb[0:H, g, :] if jt % 2 == 0
                               else k_sb[H:2 * H, g, :])
                        if (m // 2) == (jt % 2):
                            nc.vector.tensor_scalar_add(dst, src,
                                                        bias(0, sb))
                        elif cross_eng == "scalar":
                            # k0's partition-crossing copies ride the
                            # then-idle ScalarE; k1's would queue behind
                            # the exp backlog, so they go to DVE
                            nc.scalar.activation(
                                dst, src,
                                mybir.ActivationFunctionType.Identity,
                                bias=bias(0, sb))
                        else:
                            nc.vector.tensor_scalar_add(dst, src,
                                                        bias(0, sb))
                return fn

            # q unit u covers query cols [512u, 512u+512)
            def q_out(u):
                c0 = 512 * u

                def fn(pj):
                    lo = q_sb[0:H, ds(c0, CHUNK)]
                    hi = q_sb[H:2 * H, ds(c0 + CHUNK, CHUNK)]
                    nc.vector.tensor_scalar_add(lo, pj[0:H, 0:CHUNK],
                                                bias(2, 0))
                    nc.vector.tensor_scalar_add(hi, pj[H:2 * H, 0:CHUNK],
                                                bias(2, H))
                    # duplicate into the other partition half (same
                    # engine -> in-order, no cross-engine semaphore)
                    nc.vector.tensor_copy(q_sb[H:2 * H, ds(c0, CHUNK)], lo)
                    nc.vector.tensor_copy(q_sb[0:H, ds(c0 + CHUNK, CHUNK)],
                                          hi)
                return fn

            # v unit u covers local v cols [256u, 256u+256) = tiles 2u,2u+1
            def v_out(u):
                c0 = 256 * u

                def fn(pj):
                    nc.vector.tensor_scalar_add(
                        vT_sb[0:H, ds(c0, P)], pj[0:H, 0:P], bias(1, 0))
                    nc.vector.tensor_scalar_add(
                        vT_sb[0:H, ds(c0 + P, P)], pj[H:2 * H, 0:P],
                        bias(1, H))
                    for jt in (2 * u, 2 * u + 1):
                        pvt = psm.tile([P, P], BF, tag="ps", name="pvt")
                        nc.tensor.transpose(pvt, vT_sb[:, ds(jt * P, P)],
                                            identb)
                        nc.vector.tensor_copy(v_sb[:, jt, 0:H], pvt[:, :H])
                return fn

            W = 2 * CHUNK  # 512
            att_tiles = {}
            po_tiles = {}
            mask_pending = {}

            # score round r of pair pr: local tiles (2r, 2r+1) computed
            # CONCURRENTLY in the lower/upper PE row-halves.
            def emit_round(pr, r):
                diag = (r == pr)
                nb = CHUNK if diag else W      # odd tile solo on diag
                c0 = pr * W
                c0b = c0 + (CHUNK if diag else 0)
                width = W + nb
                rt = psr.tile([P, 2 * W], FP, tag="rt", name="rt")
                nc.tensor.matmul(rt[:, 0:W], lhsT=k_sb[0:H, r, :],
                                 rhs=q_sb[0:H, ds(c0, W)],
                                 start=True, stop=True)
                nc.tensor.matmul(rt[:, W:W + nb], lhsT=k_sb[H:2 * H, r, :],
                                 rhs=q_sb[H:2 * H, ds(c0b, nb)],
                                 start=True, stop=True)
                att = attp.tile([P, 2 * W], BF, tag="att", name="att")
                # k pre-scaled by 1/8 on host -> scores need no scale;
                # one exp spans the round's two PSUM banks
                nc.scalar.activation(att[:, 0:width], rt[:, 0:width],
                                     mybir.ActivationFunctionType.Exp)
                if diag:
                    mask_pending[pr] = (att, nb)
                att_tiles[(pr, 2 * r)] = (att[:, 0:W], c0, W)
                att_tiles[(pr, 2 * r + 1)] = (att[:, W:W + nb], c0b, nb)

            # causal-mask the boundary round of pair pr; deferred until
            # just before its avs so no engine FIFO is head-of-line
            # blocked waiting on the exp.  Off-tail masks ride the
            # otherwise-idle GpSimd; the last one rides the faster DVE.
            def emit_mask(pr, eng=None):
                eng = eng or nc.gpsimd
                att, nb = mask_pending.pop(pr)
                eng.tensor_mul(att[:, 0:W], att[:, 0:W], mask_sb[:, 0:W])
                eng.tensor_mul(att[:, W:W + nb], att[:, W:W + nb],
                               mask_sb[:, 0:nb])

            def emit_av(pr, jt):
                solo = KS_PAIRS[pr][1]
                if pr not in po_tiles:
                    po_tiles[pr] = pout.tile([H + 1, W], FP, tag="po",
                                             name=f"po{pr}")
                att, c0, n = att_tiles.pop((pr, jt))
                nc.tensor.matmul(po_tiles[pr][:, ds(c0 - pr * W, n)],
                                 lhsT=v_sb[:, jt, :], rhs=att,
                                 start=(jt == 0), stop=(jt == solo - 1),
                                 skip_group_check=True)

            def epilogue(pr):
                po = po_tiles[pr]
                stage = outp.tile([H + 1, W], FP, tag="stage")
                nc.vector.tensor_copy(stage, po)
                # HWDGE on the sync ring: the SWDGE descriptor build
                # costs ~2.5us, far too slow for the kernel tail
                nc.sync.dma_start(out_d[:, pr, :], stage)

            # ---- emission schedule (arrival order: wall, k0, q0, q1,
            # v0, q2, q3, k1, v1, v2, v3) ----
            proj_pair(wk_sb, xk_sb[:, 0], 512, k_out(0, "scalar"), "pk0")
            warm(5)   # bridge PE idle until q0 arrives
            proj_pair(wq_sb, xq_sb[:, 0], 512, q_out(0), "pq0")
            emit_round(0, 0)
            proj_pair(wq_sb, xq_sb[:, 1], 512, q_out(1), "pq1")
            emit_round(1, 0)
            emit_round(1, 1)
            proj_pair(wv_sb, xv_sb[:, 0], 256, v_out(0), "pv0")
            emit_mask(0, eng=nc.vector)
            # exp-gated avs are deferred BEHIND the next proj group so
            # data-ready projection matmuls never queue behind them in
            # the in-order PE FIFO; the avs then fill the PE while that
            # projection's DVE dequant chain runs
            proj_pair(wq_sb, xq_sb[:, 2], 512, q_out(2), "pq2")
            emit_av(0, 0)
            emit_av(0, 1)
            epilogue(0)
            emit_av(1, 0)
            emit_av(1, 1)
            emit_round(2, 0)
            emit_round(2, 1)
            proj_pair(wq_sb, xq_sb[:, 3], 512, q_out(3), "pq3")
            emit_av(2, 0)
            emit_av(2, 1)
            emit_round(3, 0)
            emit_round(3, 1)
            emit_mask(1)
            proj_pair(wk_sb, xk_sb[:, 1], 512, k_out(1, "vector"), "pk1")
            emit_round(2, 2)
            emit_round(3, 2)
            emit_round(3, 3)
            proj_pair(wv_sb, xv_sb[:, 1], 256, v_out(1), "pv1")
            emit_av(1, 2)
            emit_av(1, 3)
            epilogue(1)
            emit_av(2, 2)
            emit_av(2, 3)
            emit_av(3, 0)
            emit_av(3, 1)
            emit_av(3, 2)
            emit_av(3, 3)
            emit_mask(2)
            proj_pair(wv_sb, xv_sb[:, 2], 256, v_out(2), "pv2")
            emit_av(2, 4)
            emit_av(2, 5)
            epilogue(2)
            emit_av(3, 4)
            emit_av(3, 5)
            emit_mask(3, eng=nc.vector)
            proj_pair(wv_sb, xv_sb[:, 3], 256, v_out(3), "pv3")
            emit_av(3, 6)
            emit_av(3, 7)
            epilogue(3)

    nc.compile()
    return nc


def _pack(xT):
    """[D, cols] -> [128, D/128, cols]: one contiguous DMA line/partition."""
    d, s = xT.shape
    return np.ascontiguousarray(
        xT.reshape(DCH, P, s).transpose(1, 0, 2)).astype(BF_NP)


def _np_reference(query, key, value, mask, Wq, bq, Wk, bk, Wv, bv):
    q = query @ Wq + bq
    k = key @ Wk + bk
    v = value @ Wv + bv
    scores = np.einsum("bqh,bkh->bqk", q, k) / np.sqrt(np.float32(H))
    scores = np.where(mask, scores, np.float32(-1e9))
    scores -= scores.max(axis=-1, keepdims=True)
    e = np.exp(scores)
    att = e / e.sum(axis=-1, keepdims=True)
    return np.einsum("bqk,bkh->bqh", att, v).astype(np.float32)


def kernel(query, key, value, mask, Wq, bq, Wk, bk, Wv, bv):
    global LAST_RESULTS
    query = np.asarray(query, dtype=np.float32)
    key = np.asarray(key, dtype=np.float32)
    value = np.asarray(value, dtype=np.float32)
    mask = np.asarray(mask).astype(bool)
    Wq = np.asarray(Wq, dtype=np.float32)
    Wk = np.asarray(Wk, dtype=np.float32)
    Wv = np.asarray(Wv, dtype=np.float32)
    bq = np.asarray(bq, dtype=np.float32)
    bk = np.asarray(bk, dtype=np.float32)
    bv = np.asarray(bv, dtype=np.float32)

    tril = np.tril(np.ones((S, S), dtype=bool))
    if not all(np.array_equal(mask[b], tril) for b in range(B)):
        # non-causal masks never occur for this problem; fall back to an
        # exact host implementation rather than an untested device path
        return _np_reference(query, key, value, mask, Wq, bq, Wk, bk,
                             Wv, bv)

    if "rp" not in _PROGRAM_CACHE:
        _PROGRAM_CACHE["rp"] = _build_program()
    nc = _PROGRAM_CACHE["rp"]

    def packw(w):
        return np.ascontiguousarray(
            w.reshape(DCH, P, H).transpose(1, 0, 2)).astype(BF_NP)

    # weight layout must match the wall_sb slicing: wk | wv | wq.
    # k is pre-scaled by 1/8 so scores come out of the matmul pre-scaled
    # and the exp activation needs no scale parameter.
    wall_in = np.concatenate(
        [packw(Wk * 0.125), packw(Wv), packw(Wq)], axis=2)
    wall_in = np.ascontiguousarray(wall_in)
    ball_half = np.stack([bk * 0.125, bv, bq], axis=1).astype(np.float32)
    ball_in = np.ascontiguousarray(
        np.concatenate([ball_half, ball_half], axis=0))

    pvec = np.arange(P, dtype=np.float32)
    ivec = np.arange(512, dtype=np.float32)
    in_maps = []
    for c in range(NCORES):
        b, h = divmod(c, 2)
        # even/odd global-tile interleave: core h owns tiles h, h+2, ...
        jglob = list(range(h, 2 * JTL, 2))
        cols = np.concatenate(
            [np.arange(j * P, (j + 1) * P) for j in jglob])

        def units(full, w):
            n = full.shape[-1] // w
            return np.ascontiguousarray(np.stack(
                [full[:, :, w * u:w * (u + 1)] for u in range(n)], axis=1))

        kT = units(_pack(key[b][cols].T), 512)      # [P, 2, DCH, 512]
        vT = units(_pack(value[b][cols].T), 256)    # [P, 4, DCH, 256]
        qT = units(_pack(query[b].T), 512)          # [P, 4, DCH, 512]
        # causal predicate for every masked tile on this core:
        # allowed iff qcol >= p + 128*h
        mask_in = (ivec[None, :] >= pvec[:, None] + 128 * h).astype(BF_NP)
        im = {"qT": qT, "kT": kT, "vT": vT,
              "wall": wall_in, "ball": ball_in,
              "mask": np.ascontiguousarray(mask_in)}
        in_maps.append(im)

    results = run_bass_kernel_spmd(
        nc, in_maps, core_ids=list(range(NCORES)),
        trace=bool(os.environ.get("BASS_TRACE")),
    )
    LAST_RESULTS = results

    out = np.empty((B, S, H), dtype=np.float32)
    for b in range(B):
        oA = results.results[2 * b]["out"].reshape(H + 1, NQ)
        oB = results.results[2 * b + 1]["out"].reshape(H + 1, NQ)
        num = oA[:H] + oB[:H]
        den = oA[H] + oB[H]
        out[b] = (num / den).T
    return out
